# revision 1
# baseline (speedup 1.0000x reference)
"""Multi-head causal attention with RoPE on 8 TRN2 NeuronCores.

Sharding: 2 heads per core (head-parallel QKV + attention), then two
head-split AllToAlls regroup the context to t-sharded cores for the
output projection. All matmuls run in float32r (reduced-precision fp32,
1 cycle/row at N>=512 vs 4 for fp32; measured max rel err ~1.5e-4 on a
K=2048 contraction).

Layouts (per core, heads hg = 2i, 2i+1):
  qd/kd  SBUF [128=d, 2*4096]   head hl at cols [hl*4096 + t], t = b*2048+pos
                                 partitions = [even rope dims; odd rope dims]
                                 (host permutes Wq/Wk columns so this holds)
  vs     SBUF [128=t%128, 32*256] t-block tbg at cols [tbg*256 + (hl*128+dv)]
  scores S^T  PSUM [t=128, r=512] -> exp -> P^T f32r in SBUF
  ctx^T  PSUM [dv=128, r=512]     accumulated over t-blocks; denom via
                                  ones-matmul [128,512] (all rows equal)
  out^T  [oc=2048, my 512 t]      host concatenates + transposes
"""
import sys

if '/opt/trn_rl_repo' not in sys.path:
    sys.path.insert(0, '/opt/trn_rl_repo')

import numpy as np
import ml_dtypes
import concourse.bass as bass  # noqa: F401  (registers bass types)
import concourse.bacc as bacc
import concourse.mybir as mybir
import concourse.tile as tile
from concourse import bass_utils

B, T, D, H, DH = 2, 2048, 2048, 16, 128
NCORES = 8
HPC = H // NCORES          # heads per core = 2
DC = HPC * DH              # output cols per core for q/k/v = 256
BT = B * T                 # 4096
TS = 512                   # t-super / r-super tile
NTS = BT // TS             # 8
KC = D // 128              # 16 contraction chunks
NRS = T // TS              # 4 r-supers per (b, h) pair
SCALE = 1.0 / float(np.sqrt(DH))
ROPE_THETA = 10000.0

_cache = {}


def build(dbg=False):
    key = ('nc', dbg)
    if key in _cache:
        return _cache[key]
    dtr = mybir.dt.float32r
    dtf = mybir.dt.float32
    nc = bacc.Bacc("TRN2", target_bir_lowering=False, debug=False,
                   num_devices=NCORES)
    dbg_t = {}
    if dbg:
        for nm, shp in (("dqd", [128, HPC * BT]), ("dkd", [128, HPC * BT]),
                        ("dvs", [128, (BT // 128) * DC]),
                        ("dsend1", [NCORES * 128, TS]),
                        ("dsend2", [NCORES * 128, TS]),
                        ("drecv1", [NCORES * 128, TS]),
                        ("drecv2", [NCORES * 128, TS])):
            dbg_t[nm] = nc.dram_tensor(nm, shp, dtf, kind="ExternalOutput").ap()

    xT = nc.dram_tensor("xT", [D, BT], dtr, kind="ExternalInput").ap()
    wq = nc.dram_tensor("wq", [D, DC], dtr, kind="ExternalInput").ap()
    wk = nc.dram_tensor("wk", [D, DC], dtr, kind="ExternalInput").ap()
    wv = nc.dram_tensor("wv", [D, DC], dtr, kind="ExternalInput").ap()
    wo = nc.dram_tensor("wo", [D, D], dtr, kind="ExternalInput").ap()
    cosd = nc.dram_tensor("cosd", [128, T], mybir.dt.float16, kind="ExternalInput").ap()
    sind = nc.dram_tensor("sind", [128, T], mybir.dt.float16, kind="ExternalInput").ap()
    mskd = nc.dram_tensor("mskd", [128, 4 * TS], mybir.dt.bfloat16,
                          kind="ExternalInput").ap()
    onesd = nc.dram_tensor("onesd", [128, 128], dtr, kind="ExternalInput").ap()
    pswapd = nc.dram_tensor("pswapd", [128, 128], dtr, kind="ExternalInput").ap()
    outp = nc.dram_tensor("out", [D, TS], dtf, kind="ExternalOutput").ap()

    with tile.TileContext(nc) as tc:
        with tc.tile_pool(name="const", bufs=1) as constp, \
             tc.tile_pool(name="big", bufs=1) as bigp, \
             tc.tile_pool(name="xt", bufs=2) as xtp, \
             tc.tile_pool(name="rt", bufs=1) as rtp, \
             tc.tile_pool(name="pt", bufs=2) as ptp, \
             tc.tile_pool(name="dv", bufs=1) as dvp, \
             tc.tile_pool(name="cc", bufs=16) as ccp, \
             tc.tile_pool(name="wop", bufs=4) as wop, \
             tc.tile_pool(name="ot", bufs=1) as otp, \
             tc.tile_pool(name="ps", bufs=4, space="PSUM") as psp, \
             tc.tile_pool(name="ps2", bufs=2, space="PSUM") as ps2p, \
             tc.tile_pool(name="dram", bufs=1, space="DRAM") as dramp:

            # ---- constants -> SBUF ----
            wq_s = constp.tile([128, KC * DC], dtr)
            wk_s = constp.tile([128, KC * DC], dtr)
            wv_s = constp.tile([128, KC * DC], dtr)
            for dst, src in ((wq_s, wq), (wk_s, wk), (wv_s, wv)):
                nc.sync.dma_start(
                    dst[:].rearrange("p (k m) -> p k m", k=KC),
                    src.rearrange("(k p) m -> p k m", p=128))
            cos_s = constp.tile([128, T], mybir.dt.float16)
            sin_s = constp.tile([128, T], mybir.dt.float16)
            nc.sync.dma_start(cos_s[:], cosd[:, :])
            nc.sync.dma_start(sin_s[:], sind[:, :])
            msk_s = constp.tile([128, 4 * TS], mybir.dt.bfloat16)
            nc.sync.dma_start(msk_s[:], mskd[:, :])
            ones_s = constp.tile([128, 128], dtr)
            nc.sync.dma_start(ones_s[:], onesd[:, :])
            psw_s = constp.tile([128, 128], dtr)
            nc.sync.dma_start(psw_s[:], pswapd[:, :])

            qd = bigp.tile([128, HPC * BT], dtr)
            kd = bigp.tile([128, HPC * BT], dtr)
            vs = bigp.tile([128, (BT // 128) * DC], dtr)

            send1 = dramp.tile([NCORES * 128, TS], dtr)
            recv1 = dramp.tile([NCORES * 128, TS], dtr)
            send2 = dramp.tile([NCORES * 128, TS], dtr)
            recv2 = dramp.tile([NCORES * 128, TS], dtr)

            # ---- phase B: projections + RoPE. Q/K stream 512-wide (better
            # weight-load amortization); each V t-block accumulator owns its
            # own PSUM tile (interleaved accumulation groups inside one bank
            # corrupt each other) — tb 0,1 on "acc", tb 2,3 on "rot" (idle
            # during the k-loop). ----
            for ts in range(NTS):
                pos0 = (ts % NRS) * TS     # position within batch
                psq = [psp.tile([128, TS], dtf, tag="mm", name=f"psq{_h}")
                       for _h in range(2)]
                psk = [psp.tile([128, TS], dtf, tag="mm", name=f"psk{_h}")
                       for _h in range(2)]
                psv = [ps2p.tile([128, DC], dtf, tag=("acc" if _t < 2 else "rot"),
                                 name=f"psv{_t}") for _t in range(4)]
                for k in range(KC):
                    xt = xtp.tile([128, TS], dtr)
                    nc.sync.dma_start(
                        xt[:], xT[k * 128:(k + 1) * 128, ts * TS:(ts + 1) * TS])
                    st, sp = (k == 0), (k == KC - 1)
                    for hl in range(2):
                        wq_c = wq_s[:, k * DC + hl * 128: k * DC + (hl + 1) * 128]
                        wk_c = wk_s[:, k * DC + hl * 128: k * DC + (hl + 1) * 128]
                        nc.tensor.matmul(psq[hl][:], wq_c, xt[:], start=st, stop=sp)
                        nc.tensor.matmul(psk[hl][:], wk_c, xt[:], start=st, stop=sp)
                    for tb in range(4):
                        nc.tensor.matmul(
                            psv[tb][:], xt[:, tb * 128:(tb + 1) * 128],
                            wv_s[:, k * DC:(k + 1) * DC], start=st, stop=sp)
                # V eviction: [128, 256] copies into vs (t-block tbg = 4*ts+tb)
                for tb in range(4):
                    tbg = ts * 4 + tb
                    nc.scalar.copy(vs[:, tbg * DC:(tbg + 1) * DC], psv[tb][:])
                # RoPE: o = psl*cos + swap(psl)*[-sin;sin]
                for psl, dst in ((psq, qd), (psk, kd)):
                    for hl in range(2):
                        tmp = rtp.tile([128, TS], dtr)
                        nc.scalar.copy(tmp[:], psl[hl][:])
                        psr = ps2p.tile([128, TS], dtf, tag="rot")
                        nc.tensor.matmul(psr[:], psw_s[:], tmp[:],
                                         start=True, stop=True)
                        odst = dst[:, hl * BT + ts * TS: hl * BT + (ts + 1) * TS]
                        nc.vector.tensor_mul(odst, psl[hl][:],
                                             cos_s[:, pos0:pos0 + TS])
                        nc.vector.tensor_mul(psr[:], psr[:],
                                             sin_s[:, pos0:pos0 + TS])
                        nc.vector.tensor_add(odst, odst, psr[:])

            # ---- phase C: attention, pair order (b0,h0),(b1,h0) | (b0,h1),(b1,h1)
            for hl in range(2):
                for b in range(B):
                    qh0 = hl * BT + b * T
                    for R in range(NRS):
                        ps_ctx = ps2p.tile([128, TS], dtf, tag="acc")
                        ps_den = ps2p.tile([128, TS], dtf, tag="rot")
                        ntb = 4 * (R + 1)
                        for tb in range(ntb):
                            ps_s = psp.tile([128, TS], dtf, tag="mm")
                            nc.tensor.matmul(
                                ps_s[:],
                                kd[:, qh0 + tb * 128: qh0 + (tb + 1) * 128],
                                qd[:, qh0 + R * TS: qh0 + (R + 1) * TS],
                                start=True, stop=True)
                            pt = ptp.tile([128, TS], dtr)
                            nc.scalar.activation(
                                pt[:], ps_s[:], mybir.ActivationFunctionType.Exp,
                                scale=SCALE)
                            j = tb - 4 * R
                            if j >= 0:
                                nc.vector.tensor_mul(
                                    pt[:], pt[:], msk_s[:, j * TS:(j + 1) * TS])
                            tbg = b * (T // 128) + tb
                            vh = vs[:, tbg * DC + hl * 128: tbg * DC + (hl + 1) * 128]
                            st, sp = (tb == 0), (tb == ntb - 1)
                            nc.tensor.matmul(ps_ctx[:], vh, pt[:], start=st, stop=sp)
                            nc.tensor.matmul(ps_den[:], ones_s[:], pt[:],
                                             start=st, stop=sp)
                        rc = dvp.tile([128, TS], dtf)
                        nc.vector.reciprocal(rc[:], ps_den[:])
                        cx = dvp.tile([128, TS], dtr)
                        nc.vector.tensor_mul(cx[:], ps_ctx[:], rc[:])
                        jblk = b * NRS + R
                        sendb = send1 if hl == 0 else send2
                        nc.sync.dma_start(
                            sendb[jblk * 128:(jblk + 1) * 128, :], cx[:])
                # A2A for this head-slot
                sendb, recvb = (send1, recv1) if hl == 0 else (send2, recv2)
                nc.gpsimd.collective_compute(
                    "AllToAll", mybir.AluOpType.bypass,
                    replica_groups=[list(range(NCORES))],
                    ins=[sendb.opt()], outs=[recvb.opt()])

            if dbg:
                for nm, src_t in (("dqd", qd), ("dkd", kd), ("dvs", vs)):
                    nc.sync.dma_start(dbg_t[nm].bitcast(dtr), src_t[:])
                for nm, src_t in (("dsend1", send1), ("dsend2", send2),
                                  ("drecv1", recv1), ("drecv2", recv2)):
                    nc.sync.dma_start(dbg_t[nm].bitcast(dtr), src_t[:])

            # ---- phase D: output projection (t-sharded, full Wo) ----
            ctx_t = []
            for g in range(KC):       # global cd chunk = head g
                src = recv1 if g % 2 == 0 else recv2
                c = g // 2
                t_ = ccp.tile([128, TS], dtr, tag="cc")
                nc.sync.dma_start(t_[:], src[c * 128:(c + 1) * 128, :])
                ctx_t.append(t_)
            for oc in range(KC):
                ps_o = psp.tile([128, TS], dtf, tag="mm")
                for g in range(KC):
                    wo_t = wop.tile([128, 128], dtr)
                    nc.sync.dma_start(
                        wo_t[:],
                        wo[g * 128:(g + 1) * 128, oc * 128:(oc + 1) * 128])
                    nc.tensor.matmul(
                        ps_o[:], wo_t[:], ctx_t[g][:],
                        start=(g == 0), stop=(g == KC - 1))
                ot = otp.tile([128, TS], dtf)
                nc.scalar.copy(ot[:], ps_o[:])
                nc.sync.dma_start(outp[oc * 128:(oc + 1) * 128, :], ot[:])

    nc.compile()
    _cache[key] = nc
    return nc


def host_prep(x, Wq, Wk, Wv, Wo):
    x = np.asarray(x, dtype=np.float32)
    Wq = np.asarray(Wq, dtype=np.float32)
    Wk = np.asarray(Wk, dtype=np.float32)
    Wv = np.asarray(Wv, dtype=np.float32)
    Wo = np.asarray(Wo, dtype=np.float32)

    xT = np.ascontiguousarray(x.reshape(BT, D).T)
    perm = np.concatenate([np.arange(0, DH, 2), np.arange(1, DH, 2)])

    pos = np.arange(T, dtype=np.float64)
    inv = ROPE_THETA ** (-np.arange(0, DH, 2, dtype=np.float64) / DH)  # [64]
    ang = inv[:, None] * pos[None, :]                                  # [64, T]
    c64 = np.cos(ang)
    s64 = np.sin(ang)
    cos128 = np.concatenate([c64, c64], axis=0).astype(np.float16)   # [128, T]
    sin128 = np.concatenate([-s64, s64], axis=0).astype(np.float16)  # [-sin; sin]

    tl = np.arange(128)[:, None]
    rl = np.arange(TS)[None, :]
    msk = np.concatenate(
        [(tl + 128 * j <= rl).astype(np.float32) for j in range(4)],
        axis=1).astype(ml_dtypes.bfloat16)                     # [128, 4*TS]

    ones = np.ones((128, 128), dtype=np.float32)
    pswap = np.zeros((128, 128), dtype=np.float32)
    pswap[(np.arange(128) + 64) % 128, np.arange(128)] = 1.0

    in_maps = []
    for i in range(NCORES):
        idx = np.concatenate([i * DC + h * DH + perm for h in range(HPC)])
        in_maps.append({
            "xT": xT,
            "wq": np.ascontiguousarray(Wq[:, idx]),
            "wk": np.ascontiguousarray(Wk[:, idx]),
            "wv": np.ascontiguousarray(Wv[:, i * DC:(i + 1) * DC]),
            "wo": Wo,
            "cosd": cos128, "sind": sin128, "mskd": msk,
            "onesd": ones, "pswapd": pswap,
        })
    return in_maps


def assemble(results):
    out_T = np.concatenate([results[i]["out"] for i in range(NCORES)], axis=1)
    return np.ascontiguousarray(out_T.T).reshape(B, T, D).astype(np.float32)


def kernel(x, Wq, Wk, Wv, Wo):
    nc = build()
    in_maps = host_prep(x, Wq, Wk, Wv, Wo)
    r = bass_utils.run_bass_kernel_spmd(nc, in_maps,
                                        core_ids=list(range(NCORES)))
    return assemble(r.results)



# revision 10
# speedup vs baseline: 3.7088x; 3.7088x over previous
"""Multi-head causal attention with RoPE on 8 TRN2 NeuronCores.

Sharding: 2 heads per core (head-parallel QKV + attention), then two
head-split AllToAlls regroup the context to t-sharded cores for the
output projection. v2: all SBUF operands bf16 (halves DMA + collective
bytes; matmul rate is 1 cycle/row for bf16 = same as f32r; measured
end-to-end rel err 3.2e-3 vs the 2e-2 gate), batched DMAs (4 per 512-t
super instead of 16; per-oc Wo tiles instead of 256 singles), cx sends
on the SWDGE queue so the AllToAlls don't entangle with unrelated HWDGE
semaphore counts, and the output projection split into even-head/
odd-head halves so the even half (fed by AllToAll #1) hides AllToAll #2.

Layouts (per core, heads hg = 2i, 2i+1):
  qd/kd  SBUF bf16 [128=d, 2*4096]  head hl at cols [hl*4096 + t]
                                    partitions = [even rope dims; odd]
                                    (host permutes Wq/Wk columns)
  vs     SBUF bf16 [128=t%128, 32*256] t-block tbg at cols [tbg*256+(hl*128+dv)]
  scores S^T PSUM [t=128, r=512] -> exp -> P^T bf16 in SBUF
  ctx^T  PSUM [dv=128, r=512]  accumulated over t-blocks; denom via
                               ones-matmul [128,512] (all rows equal)
  out^T  [oc=2048, my 512 t]   host concatenates + transposes
"""
import sys

if '/opt/trn_rl_repo' not in sys.path:
    sys.path.insert(0, '/opt/trn_rl_repo')

import numpy as np
import ml_dtypes
import concourse.bass as bass  # noqa: F401  (registers bass types)
import concourse.bacc as bacc
import concourse.mybir as mybir
import concourse.tile as tile
from concourse import bass_utils

B, T, D, H, DH = 2, 2048, 2048, 16, 128
NCORES = 8
HPC = H // NCORES          # heads per core = 2
DC = HPC * DH              # output cols per core for q/k/v = 256
BT = B * T                 # 4096
TS = 512                   # t-super / r-super tile
NTS = BT // TS             # 8
KC = D // 128              # 16 contraction chunks
NRS = T // TS              # 4 r-supers per (b, h) pair
HG = KC // 2               # 8 head-groups per parity in outproj
SCALE = 1.0 / float(np.sqrt(DH))
ROPE_THETA = 10000.0

_cache = {}


def build(reps=1):
    """reps>1 unrolls the whole pipeline N times inside one NEFF (constants
    loaded once) — used by test.py to measure per-iteration HW time as a
    slope, free of per-call dispatch/staging noise."""
    key = ('nc', reps)
    if key in _cache:
        return _cache[key]
    dtb = mybir.dt.bfloat16
    dtf = mybir.dt.float32
    nc = bacc.Bacc("TRN2", target_bir_lowering=False, debug=False,
                   num_devices=NCORES)

    xT = nc.dram_tensor("xT", [D, BT], dtb, kind="ExternalInput").ap()
    wq = nc.dram_tensor("wq", [D, DC], dtb, kind="ExternalInput").ap()
    wk = nc.dram_tensor("wk", [D, DC], dtb, kind="ExternalInput").ap()
    wv = nc.dram_tensor("wv", [D, DC], dtb, kind="ExternalInput").ap()
    # Wo pre-split by head parity: rows of even heads then odd heads
    woE = nc.dram_tensor("woE", [D // 2, D], dtb, kind="ExternalInput").ap()
    woO = nc.dram_tensor("woO", [D // 2, D], dtb, kind="ExternalInput").ap()
    cosd = nc.dram_tensor("cosd", [128, T], mybir.dt.float16, kind="ExternalInput").ap()
    sind = nc.dram_tensor("sind", [128, T], mybir.dt.float16, kind="ExternalInput").ap()
    mskd = nc.dram_tensor("mskd", [128, 4 * TS], dtb, kind="ExternalInput").ap()
    onesd = nc.dram_tensor("onesd", [128, 128], dtb, kind="ExternalInput").ap()
    pswapd = nc.dram_tensor("pswapd", [128, 128], dtb, kind="ExternalInput").ap()
    outp = nc.dram_tensor("out", [D, TS], dtf, kind="ExternalOutput").ap()

    with tile.TileContext(nc) as tc:
        with tc.tile_pool(name="const", bufs=1) as constp, \
             tc.tile_pool(name="big", bufs=1) as bigp, \
             tc.tile_pool(name="xt", bufs=2) as xtp, \
             tc.tile_pool(name="rt", bufs=1) as rtp, \
             tc.tile_pool(name="pt", bufs=2) as ptp, \
             tc.tile_pool(name="dv", bufs=1) as dvp, \
             tc.tile_pool(name="cc", bufs=8) as ccp, \
             tc.tile_pool(name="wop", bufs=3) as wop, \
             tc.tile_pool(name="po", bufs=1) as pop, \
             tc.tile_pool(name="ot", bufs=2) as otp, \
             tc.tile_pool(name="ps", bufs=4, space="PSUM") as psp, \
             tc.tile_pool(name="ps2", bufs=2, space="PSUM") as ps2p, \
             tc.tile_pool(name="dram", bufs=1, space="DRAM") as dramp:

            # ---- constants -> SBUF (Activation HWDGE queue, so the x-tile
            # stream on the SP queue starts in parallel) ----
            wq_s = constp.tile([128, KC * DC], dtb)
            wk_s = constp.tile([128, KC * DC], dtb)
            wv_s = constp.tile([128, KC * DC], dtb)
            for dst, src in ((wq_s, wq), (wk_s, wk), (wv_s, wv)):
                nc.scalar.dma_start(
                    dst[:].rearrange("p (k m) -> p k m", k=KC),
                    src.rearrange("(k p) m -> p k m", p=128))
            cos_s = constp.tile([128, T], mybir.dt.float16)
            sin_s = constp.tile([128, T], mybir.dt.float16)
            nc.scalar.dma_start(cos_s[:], cosd[:, :])
            nc.scalar.dma_start(sin_s[:], sind[:, :])
            msk_s = constp.tile([128, 4 * TS], dtb)
            nc.scalar.dma_start(msk_s[:], mskd[:, :])
            ones_s = constp.tile([128, 128], dtb)
            nc.scalar.dma_start(ones_s[:], onesd[:, :])
            psw_s = constp.tile([128, 128], dtb)
            nc.scalar.dma_start(psw_s[:], pswapd[:, :])

            qd = bigp.tile([128, HPC * BT], dtb)
            kd = bigp.tile([128, HPC * BT], dtb)
            vs = bigp.tile([128, (BT // 128) * DC], dtb)

            send1 = dramp.tile([NCORES * 128, TS], dtb)
            recv1 = dramp.tile([NCORES * 128, TS], dtb)
            send2 = dramp.tile([NCORES * 128, TS], dtb)
            recv2 = dramp.tile([NCORES * 128, TS], dtb)

            for _rep in range(reps):
                _one_iter(nc, tc, locals())

    nc.compile()
    _cache[key] = nc
    return nc


def _one_iter(nc, tc, env):
    dtb = mybir.dt.bfloat16
    dtf = mybir.dt.float32
    xT = env['xT']; woE = env['woE']; woO = env['woO']; outp = env['outp']
    wq_s = env['wq_s']; wk_s = env['wk_s']; wv_s = env['wv_s']
    cos_s = env['cos_s']; sin_s = env['sin_s']; msk_s = env['msk_s']
    ones_s = env['ones_s']; psw_s = env['psw_s']
    qd = env['qd']; kd = env['kd']; vs = env['vs']
    send1 = env['send1']; recv1 = env['recv1']
    send2 = env['send2']; recv2 = env['recv2']
    xtp = env['xtp']; rtp = env['rtp']; ptp = env['ptp']; dvp = env['dvp']
    ccp = env['ccp']; wop = env['wop']; pop = env['pop']; otp = env['otp']
    psp = env['psp']; ps2p = env['ps2p']

    # ---- phase B: projections + RoPE. Four x-DMA chunks per t-super (the
    # first matmuls start as soon as chunk 0 lands). Each V t-block
    # accumulator owns its own PSUM tile (interleaved accumulation groups
    # inside one bank corrupt each other) — tb 0,1 on "acc", tb 2,3 on
    # "rot" (idle during the k-loop). ----
    for ts in range(NTS):
        pos0 = (ts % NRS) * TS     # position within batch
        xt = xtp.tile([128, KC * TS], dtb, name="xt")
        for q4 in range(4):
            k0 = q4 * (KC // 4)
            k1 = (q4 + 1) * (KC // 4)
            nc.sync.dma_start(
                xt[:, k0 * TS:k1 * TS].rearrange(
                    "p (k m) -> p k m", k=KC // 4),
                xT[k0 * 128:k1 * 128, ts * TS:(ts + 1) * TS].rearrange(
                    "(k p) m -> p k m", p=128))
        psq = [psp.tile([128, TS], dtf, tag="mm", name=f"psq{_h}")
               for _h in range(2)]
        psk = [psp.tile([128, TS], dtf, tag="mm", name=f"psk{_h}")
               for _h in range(2)]
        psv = [ps2p.tile([128, DC], dtf, tag=("acc" if _t < 2 else "rot"),
                         name=f"psv{_t}") for _t in range(4)]
        for k in range(KC):
            xk = xt[:, k * TS:(k + 1) * TS]
            st, sp = (k == 0), (k == KC - 1)
            for hl in range(2):
                wq_c = wq_s[:, k * DC + hl * 128: k * DC + (hl + 1) * 128]
                wk_c = wk_s[:, k * DC + hl * 128: k * DC + (hl + 1) * 128]
                nc.tensor.matmul(psq[hl][:], wq_c, xk, start=st, stop=sp)
                nc.tensor.matmul(psk[hl][:], wk_c, xk, start=st, stop=sp)
            for tb in range(4):
                nc.tensor.matmul(
                    psv[tb][:], xt[:, k * TS + tb * 128: k * TS + (tb + 1) * 128],
                    wv_s[:, k * DC:(k + 1) * DC], start=st, stop=sp)
        # V eviction: [128, 256] copies into vs (t-block tbg = 4*ts+tb)
        for tb in range(4):
            tbg = ts * 4 + tb
            nc.scalar.copy(vs[:, tbg * DC:(tbg + 1) * DC], psv[tb][:])
        # RoPE: o = psl*cos + swap(psl)*[-sin;sin]
        for psl, dst in ((psq, qd), (psk, kd)):
            for hl in range(2):
                tmp = rtp.tile([128, TS], dtb, name="tmp")
                nc.scalar.copy(tmp[:], psl[hl][:])
                psr = ps2p.tile([128, TS], dtf, tag="rot", name="psr")
                nc.tensor.matmul(psr[:], psw_s[:], tmp[:],
                                 start=True, stop=True)
                odst = dst[:, hl * BT + ts * TS: hl * BT + (ts + 1) * TS]
                nc.vector.tensor_mul(odst, psl[hl][:],
                                     cos_s[:, pos0:pos0 + TS])
                nc.vector.tensor_mul(psr[:], psr[:],
                                     sin_s[:, pos0:pos0 + TS])
                nc.vector.tensor_add(odst, odst, psr[:])

    # ---- phase C: attention, pair order (b0,h0),(b1,h0) | (b0,h1),(b1,h1)
    for hl in range(2):
        for b in range(B):
            qh0 = hl * BT + b * T
            for R in range(NRS):
                ps_ctx = ps2p.tile([128, TS], dtf, tag="acc", name="ps_ctx")
                ps_den = ps2p.tile([128, TS], dtf, tag="rot", name="ps_den")
                ntb = 4 * (R + 1)
                for tb in range(ntb):
                    ps_s = psp.tile([128, TS], dtf, tag="mm", name="ps_s")
                    nc.tensor.matmul(
                        ps_s[:],
                        kd[:, qh0 + tb * 128: qh0 + (tb + 1) * 128],
                        qd[:, qh0 + R * TS: qh0 + (R + 1) * TS],
                        start=True, stop=True)
                    pt = ptp.tile([128, TS], dtb, name="pt")
                    nc.scalar.activation(
                        pt[:], ps_s[:], mybir.ActivationFunctionType.Exp,
                        scale=SCALE)
                    j = tb - 4 * R
                    if j >= 0:
                        nc.vector.tensor_mul(
                            pt[:], pt[:], msk_s[:, j * TS:(j + 1) * TS])
                    tbg = b * (T // 128) + tb
                    vh = vs[:, tbg * DC + hl * 128: tbg * DC + (hl + 1) * 128]
                    st, sp = (tb == 0), (tb == ntb - 1)
                    nc.tensor.matmul(ps_ctx[:], vh, pt[:], start=st, stop=sp)
                    nc.tensor.matmul(ps_den[:], ones_s[:], pt[:],
                                     start=st, stop=sp)
                rc = dvp.tile([128, TS], dtf, name="rc")
                nc.vector.reciprocal(rc[:], ps_den[:])
                cx = dvp.tile([128, TS], dtb, name="cx")
                nc.vector.tensor_mul(cx[:], ps_ctx[:], rc[:])
                jblk = b * NRS + R
                sendb = send1 if hl == 0 else send2
                # SWDGE (gpsimd) queue: the AllToAll then waits only on its
                # own sends' DMASW semaphores, not on every HWDGE DMA the
                # scheduler placed before it.
                nc.gpsimd.dma_start(
                    sendb[jblk * 128:(jblk + 1) * 128, :], cx[:])
        # A2A for this head-slot
        sendb, recvb = (send1, recv1) if hl == 0 else (send2, recv2)
        nc.gpsimd.collective_compute(
            "AllToAll", mybir.AluOpType.bypass,
            replica_groups=[list(range(NCORES))],
            ins=[sendb.opt()], outs=[recvb.opt()])

    # ---- phase D: output projection (t-sharded, full Wo), split by head
    # parity: the even-head half (fed by AllToAll #1) runs while AllToAll
    # #2 is in flight; partials parked in SBUF f32. ----
    po = [pop.tile([128, TS], dtf, name=f"po{oc}") for oc in range(KC)]
    for half, (recvb, wohalf) in enumerate(((recv1, woE), (recv2, woO))):
        ctx_t = []
        for j in range(HG):
            t_ = ccp.tile([128, TS], dtb, tag="cc", name="ctx")
            nc.sync.dma_start(t_[:], recvb[j * 128:(j + 1) * 128, :])
            ctx_t.append(t_)
        for oc in range(KC):
            wo_t = wop.tile([128, HG * 128], dtb, name="wo_t")
            # Act HWDGE queue: independent of the A2A-gated ctx DMAs on the
            # SP queue (no head-of-line blocking).
            nc.scalar.dma_start(
                wo_t[:].rearrange("p (j m) -> p j m", j=HG),
                wohalf[:, oc * 128:(oc + 1) * 128].rearrange(
                    "(j p) m -> p j m", p=128))
            ps_o = psp.tile([128, TS], dtf, tag="mm", name="ps_o")
            for j in range(HG):
                nc.tensor.matmul(
                    ps_o[:], wo_t[:, j * 128:(j + 1) * 128], ctx_t[j][:],
                    start=(j == 0), stop=(j == HG - 1))
            if half == 0:
                nc.scalar.copy(po[oc][:], ps_o[:])
            else:
                ot = otp.tile([128, TS], dtf, name="ot")
                nc.vector.tensor_add(ot[:], ps_o[:], po[oc][:])
                nc.sync.dma_start(outp[oc * 128:(oc + 1) * 128, :], ot[:])


def host_prep(x, Wq, Wk, Wv, Wo):
    bf = ml_dtypes.bfloat16
    x = np.asarray(x, dtype=np.float32)
    Wq = np.asarray(Wq, dtype=np.float32)
    Wk = np.asarray(Wk, dtype=np.float32)
    Wv = np.asarray(Wv, dtype=np.float32)
    Wo = np.asarray(Wo, dtype=np.float32)

    xT = np.ascontiguousarray(x.reshape(BT, D).T).astype(bf)
    perm = np.concatenate([np.arange(0, DH, 2), np.arange(1, DH, 2)])

    pos = np.arange(T, dtype=np.float64)
    inv = ROPE_THETA ** (-np.arange(0, DH, 2, dtype=np.float64) / DH)  # [64]
    ang = inv[:, None] * pos[None, :]                                  # [64, T]
    c64 = np.cos(ang)
    s64 = np.sin(ang)
    cos128 = np.concatenate([c64, c64], axis=0).astype(np.float16)   # [128, T]
    sin128 = np.concatenate([-s64, s64], axis=0).astype(np.float16)  # [-sin; sin]

    tl = np.arange(128)[:, None]
    rl = np.arange(TS)[None, :]
    msk = np.concatenate(
        [(tl + 128 * j <= rl).astype(np.float32) for j in range(4)],
        axis=1).astype(bf)                                     # [128, 4*TS]

    ones = np.ones((128, 128), dtype=bf)
    pswap = np.zeros((128, 128), dtype=np.float32)
    pswap[(np.arange(128) + 64) % 128, np.arange(128)] = 1.0
    pswap = pswap.astype(bf)

    woE = np.ascontiguousarray(
        np.concatenate([Wo[(2 * j) * DH:(2 * j + 1) * DH] for j in range(HG)],
                       axis=0)).astype(bf)
    woO = np.ascontiguousarray(
        np.concatenate([Wo[(2 * j + 1) * DH:(2 * j + 2) * DH] for j in range(HG)],
                       axis=0)).astype(bf)

    in_maps = []
    for i in range(NCORES):
        idx = np.concatenate([i * DC + h * DH + perm for h in range(HPC)])
        in_maps.append({
            "xT": xT,
            "wq": np.ascontiguousarray(Wq[:, idx]).astype(bf),
            "wk": np.ascontiguousarray(Wk[:, idx]).astype(bf),
            "wv": np.ascontiguousarray(Wv[:, i * DC:(i + 1) * DC]).astype(bf),
            "woE": woE, "woO": woO,
            "cosd": cos128, "sind": sin128, "mskd": msk,
            "onesd": ones, "pswapd": pswap,
        })
    return in_maps


def assemble(results):
    out_T = np.concatenate([results[i]["out"] for i in range(NCORES)], axis=1)
    return np.ascontiguousarray(out_T.T).reshape(B, T, D).astype(np.float32)


def kernel(x, Wq, Wk, Wv, Wo):
    nc = build()
    in_maps = host_prep(x, Wq, Wk, Wv, Wo)
    r = bass_utils.run_bass_kernel_spmd(nc, in_maps,
                                        core_ids=list(range(NCORES)))
    return assemble(r.results)


# revision 17
# speedup vs baseline: 8.5153x; 2.2960x over previous
"""Multi-head causal attention with RoPE on 8 TRN2 NeuronCores.

Sharding: 2 heads per core (head-parallel QKV + attention), then two
head-split AllToAlls regroup the context to t-sharded cores for the
output projection. v2: all SBUF operands bf16 (halves DMA + collective
bytes; matmul rate is 1 cycle/row for bf16 = same as f32r; measured
end-to-end rel err 3.2e-3 vs the 2e-2 gate), batched DMAs (4 per 512-t
super instead of 16; per-oc Wo tiles instead of 256 singles), cx sends
on the SWDGE queue so the AllToAlls don't entangle with unrelated HWDGE
semaphore counts, and the output projection split into even-head/
odd-head halves so the even half (fed by AllToAll #1) hides AllToAll #2.

Layouts (per core, heads hg = 2i, 2i+1):
  qd/kd  SBUF bf16 [128=d, 2*4096]  head hl at cols [hl*4096 + t]
                                    partitions = [even rope dims; odd]
                                    (host permutes Wq/Wk columns)
  vs     SBUF bf16 [128=t%128, 32*256] t-block tbg at cols [tbg*256+(hl*128+dv)]
  scores S^T PSUM [t=128, r=512] -> exp -> P^T bf16 in SBUF
  ctx^T  PSUM [dv=128, r=512]  accumulated over t-blocks; denom via
                               ones-matmul [128,512] (all rows equal)
  out^T  [oc=2048, my 512 t]   host concatenates + transposes
"""
import sys

if '/opt/trn_rl_repo' not in sys.path:
    sys.path.insert(0, '/opt/trn_rl_repo')

import numpy as np
import ml_dtypes
import concourse.bass as bass  # noqa: F401  (registers bass types)
import concourse.bacc as bacc
import concourse.mybir as mybir
import concourse.tile as tile
from concourse import bass_utils

B, T, D, H, DH = 2, 2048, 2048, 16, 128
NCORES = 8
HPC = H // NCORES          # heads per core = 2
DC = HPC * DH              # output cols per core for q/k/v = 256
BT = B * T                 # 4096
TS = 512                   # t-super / r-super tile
NTS = BT // TS             # 8
KC = D // 128              # 16 contraction chunks
NRS = T // TS              # 4 r-supers per (b, h) pair
HG = KC // 2               # 8 head-groups per parity in outproj
SCALE = 1.0 / float(np.sqrt(DH))
ROPE_THETA = 10000.0

_cache = {}


def build(reps=1):
    """reps>1 unrolls the whole pipeline N times inside one NEFF (constants
    loaded once) — used by test.py to measure per-iteration HW time as a
    slope, free of per-call dispatch/staging noise."""
    key = ('nc', reps)
    if key in _cache:
        return _cache[key]
    dtb = mybir.dt.bfloat16
    dtf = mybir.dt.float32
    nc = bacc.Bacc("TRN2", target_bir_lowering=False, debug=False,
                   num_devices=NCORES)

    xT = nc.dram_tensor("xT", [D, BT], dtb, kind="ExternalInput").ap()
    wq = nc.dram_tensor("wq", [D, DC], dtb, kind="ExternalInput").ap()
    wk = nc.dram_tensor("wk", [D, DC], dtb, kind="ExternalInput").ap()
    wv = nc.dram_tensor("wv", [D, DC], dtb, kind="ExternalInput").ap()
    # Wo pre-split by head parity: rows of even heads then odd heads
    woE = nc.dram_tensor("woE", [D // 2, D], dtb, kind="ExternalInput").ap()
    woO = nc.dram_tensor("woO", [D // 2, D], dtb, kind="ExternalInput").ap()
    cosd = nc.dram_tensor("cosd", [128, T], mybir.dt.float16, kind="ExternalInput").ap()
    sind = nc.dram_tensor("sind", [128, T], mybir.dt.float16, kind="ExternalInput").ap()
    mskd = nc.dram_tensor("mskd", [128, 4 * TS], dtb, kind="ExternalInput").ap()
    onesd = nc.dram_tensor("onesd", [128, 128], dtb, kind="ExternalInput").ap()
    pswapd = nc.dram_tensor("pswapd", [128, 128], dtb, kind="ExternalInput").ap()
    outp = nc.dram_tensor("out", [D, TS], dtf, kind="ExternalOutput").ap()

    with tile.TileContext(nc) as tc:
        with tc.tile_pool(name="const", bufs=1) as constp, \
             tc.tile_pool(name="big", bufs=1) as bigp, \
             tc.tile_pool(name="xt", bufs=2) as xtp, \
             tc.tile_pool(name="rt", bufs=1) as rtp, \
             tc.tile_pool(name="pt", bufs=2) as ptp, \
             tc.tile_pool(name="dv", bufs=1) as dvp, \
             tc.tile_pool(name="cc", bufs=8) as ccp, \
             tc.tile_pool(name="wop", bufs=3) as wop, \
             tc.tile_pool(name="po", bufs=1) as pop, \
             tc.tile_pool(name="ot", bufs=2) as otp, \
             tc.tile_pool(name="ps", bufs=4, space="PSUM") as psp, \
             tc.tile_pool(name="ps2", bufs=2, space="PSUM") as ps2p, \
             tc.tile_pool(name="dram", bufs=1, space="DRAM") as dramp:

            # ---- constants -> SBUF (Activation HWDGE queue, so the x-tile
            # stream on the SP queue starts in parallel) ----
            wq_s = constp.tile([128, KC * DC], dtb)
            wk_s = constp.tile([128, KC * DC], dtb)
            wv_s = constp.tile([128, KC * DC], dtb)
            for dst, src in ((wq_s, wq), (wk_s, wk), (wv_s, wv)):
                for h2 in range(2):      # halves: first k-chunks land sooner
                    kh = KC // 2
                    nc.scalar.dma_start(
                        dst[:, h2 * kh * DC:(h2 + 1) * kh * DC].rearrange(
                            "p (k m) -> p k m", k=kh),
                        src[h2 * kh * 128:(h2 + 1) * kh * 128, :].rearrange(
                            "(k p) m -> p k m", p=128))
            cos_s = constp.tile([128, T], mybir.dt.float16)
            sin_s = constp.tile([128, T], mybir.dt.float16)
            nc.scalar.dma_start(cos_s[:], cosd[:, :])
            nc.scalar.dma_start(sin_s[:], sind[:, :])
            msk_s = constp.tile([128, 4 * TS], dtb)
            nc.scalar.dma_start(msk_s[:], mskd[:, :])
            ones_s = constp.tile([128, 128], dtb)
            nc.scalar.dma_start(ones_s[:], onesd[:, :])
            psw_s = constp.tile([128, 128], dtb)
            nc.scalar.dma_start(psw_s[:], pswapd[:, :])

            qd = bigp.tile([128, HPC * BT], dtb)
            kd = bigp.tile([128, HPC * BT], dtb)
            vs = bigp.tile([128, (BT // 128) * DC], dtb)

            send1 = dramp.tile([NCORES * 128, TS], dtb)
            recv1 = dramp.tile([NCORES * 128, TS], dtb)
            send2 = dramp.tile([NCORES * 128, TS], dtb)
            recv2 = dramp.tile([NCORES * 128, TS], dtb)

            for _rep in range(reps):
                _one_iter(nc, tc, locals())

    nc.compile()
    _cache[key] = nc
    return nc


def _one_iter(nc, tc, env):
    dtb = mybir.dt.bfloat16
    dtf = mybir.dt.float32
    xT = env['xT']; woE = env['woE']; woO = env['woO']; outp = env['outp']
    wq_s = env['wq_s']; wk_s = env['wk_s']; wv_s = env['wv_s']
    cos_s = env['cos_s']; sin_s = env['sin_s']; msk_s = env['msk_s']
    ones_s = env['ones_s']; psw_s = env['psw_s']
    qd = env['qd']; kd = env['kd']; vs = env['vs']
    send1 = env['send1']; recv1 = env['recv1']
    send2 = env['send2']; recv2 = env['recv2']
    xtp = env['xtp']; rtp = env['rtp']; ptp = env['ptp']; dvp = env['dvp']
    ccp = env['ccp']; wop = env['wop']; pop = env['pop']; otp = env['otp']
    psp = env['psp']; ps2p = env['ps2p']

    # ---- phase B: projections + RoPE. Four x-DMA chunks per t-super (the
    # first matmuls start as soon as chunk 0 lands). Each V t-block
    # accumulator owns its own PSUM tile (interleaved accumulation groups
    # inside one bank corrupt each other) — tb 0,1 on "acc", tb 2,3 on
    # "rot" (idle during the k-loop). ----
    for ts in range(NTS):
        pos0 = (ts % NRS) * TS     # position within batch
        xt = xtp.tile([128, KC * TS], dtb, name="xt")
        for q4 in range(4):
            k0 = q4 * (KC // 4)
            k1 = (q4 + 1) * (KC // 4)
            nc.sync.dma_start(
                xt[:, k0 * TS:k1 * TS].rearrange(
                    "p (k m) -> p k m", k=KC // 4),
                xT[k0 * 128:k1 * 128, ts * TS:(ts + 1) * TS].rearrange(
                    "(k p) m -> p k m", p=128))
        psq = [psp.tile([128, TS], dtf, tag="mm", name=f"psq{_h}")
               for _h in range(2)]
        psk = [psp.tile([128, TS], dtf, tag="mm", name=f"psk{_h}")
               for _h in range(2)]
        psv = [ps2p.tile([128, DC], dtf, tag=("acc" if _t < 2 else "rot"),
                         name=f"psv{_t}") for _t in range(4)]
        for k in range(KC):
            xk = xt[:, k * TS:(k + 1) * TS]
            st, sp = (k == 0), (k == KC - 1)
            for hl in range(2):
                wq_c = wq_s[:, k * DC + hl * 128: k * DC + (hl + 1) * 128]
                wk_c = wk_s[:, k * DC + hl * 128: k * DC + (hl + 1) * 128]
                nc.tensor.matmul(psq[hl][:], wq_c, xk, start=st, stop=sp)
                nc.tensor.matmul(psk[hl][:], wk_c, xk, start=st, stop=sp)
            for tb in range(4):
                nc.tensor.matmul(
                    psv[tb][:], xt[:, k * TS + tb * 128: k * TS + (tb + 1) * 128],
                    wv_s[:, k * DC:(k + 1) * DC], start=st, stop=sp)
        # Epilogue: RoPE + V eviction, interleaved so PSUM slots free in the
        # order the next t-super claims them (q0,k0,q1,k1 / v0..v3).
        # RoPE: o = psl*cos + swap(psl)*[-sin;sin].  The 64-partition swap
        # runs as two SBUF->SBUF DMAs off the evicted bf16 tile (cheaper
        # than a PE permutation matmul; engines can't partition-shift).
        rope_seq = [(psq[0], qd, 0), (psk[0], kd, 0),
                    (psq[1], qd, 1), (psk[1], kd, 1)]
        for ri, (psl_h, dst, hl) in enumerate(rope_seq):
            tmp = rtp.tile([128, TS], dtb, name="tmp")
            nc.scalar.copy(tmp[:], psl_h[:])
            tbg = ts * 4 + ri
            nc.scalar.copy(vs[:, tbg * DC:(tbg + 1) * DC], psv[ri][:])
            swp = rtp.tile([128, TS], dtb, name="swp")
            nc.sync.dma_start(swp[0:64, :], tmp[64:128, :])
            nc.sync.dma_start(swp[64:128, :], tmp[0:64, :])
            odst = dst[:, hl * BT + ts * TS: hl * BT + (ts + 1) * TS]
            nc.vector.tensor_mul(odst, psl_h[:],
                                 cos_s[:, pos0:pos0 + TS])
            sw2 = rtp.tile([128, TS], dtb, name="sw2")
            nc.vector.tensor_mul(sw2[:], swp[:],
                                 sin_s[:, pos0:pos0 + TS])
            nc.vector.tensor_add(odst, odst, sw2[:])

    # ---- phase C: attention, pair order (b0,h0),(b1,h0) | (b0,h1),(b1,h1)
    for hl in range(2):
        for b in range(B):
            qh0 = hl * BT + b * T
            for R in range(NRS):
                ps_ctx = ps2p.tile([128, TS], dtf, tag="acc", name="ps_ctx")
                ps_den = ps2p.tile([128, TS], dtf, tag="rot", name="ps_den")
                ntb = 4 * (R + 1)
                # All scores+exp first, PV/denominator after: while the Act
                # engine's exp chain chases the scores, the PE retires the
                # PREVIOUS super's PV/den instead of stalling block-by-block.
                # Diagonal t-block j only reaches r >= 128j within the super
                # (causality): compute the [c0:TS] r-subrange only. Column
                # ranges accumulate partially in PSUM; tb==0 is always full
                # width, so start=True initializes every column.
                pts = []
                for tb in range(ntb):
                    j = tb - 4 * R
                    c0 = 128 * j if j > 0 else 0
                    ps_s = psp.tile([128, TS], dtf, tag="mm", name="ps_s")
                    nc.tensor.matmul(
                        ps_s[:, c0:TS],
                        kd[:, qh0 + tb * 128: qh0 + (tb + 1) * 128],
                        qd[:, qh0 + R * TS + c0: qh0 + (R + 1) * TS],
                        start=True, stop=True)
                    pt = ptp.tile([128, TS], dtb, name="pt", bufs=24)
                    nc.scalar.activation(
                        pt[:, c0:TS], ps_s[:, c0:TS],
                        mybir.ActivationFunctionType.Exp, scale=SCALE)
                    if j >= 0:
                        nc.vector.tensor_mul(
                            pt[:, c0:TS], pt[:, c0:TS],
                            msk_s[:, j * TS + c0:(j + 1) * TS])
                    pts.append((c0, pt))
                for tb, (c0, pt) in enumerate(pts):
                    tbg = b * (T // 128) + tb
                    vh = vs[:, tbg * DC + hl * 128: tbg * DC + (hl + 1) * 128]
                    st, sp = (tb == 0), (tb == ntb - 1)
                    nc.tensor.matmul(ps_ctx[:, c0:TS], vh, pt[:, c0:TS],
                                     start=st, stop=sp)
                    nc.tensor.matmul(ps_den[:, c0:TS], ones_s[:], pt[:, c0:TS],
                                     start=st, stop=sp)
                rc = dvp.tile([128, TS], dtf, name="rc")
                nc.vector.reciprocal(rc[:], ps_den[:])
                cx = dvp.tile([128, TS], dtb, name="cx")
                nc.vector.tensor_mul(cx[:], ps_ctx[:], rc[:])
                jblk = b * NRS + R
                sendb = send1 if hl == 0 else send2
                # SWDGE (gpsimd) queue: the AllToAll then waits only on its
                # own sends' DMASW semaphores, not on every HWDGE DMA the
                # scheduler placed before it.
                nc.gpsimd.dma_start(
                    sendb[jblk * 128:(jblk + 1) * 128, :], cx[:])
        # A2A for this head-slot
        sendb, recvb = (send1, recv1) if hl == 0 else (send2, recv2)
        nc.gpsimd.collective_compute(
            "AllToAll", mybir.AluOpType.bypass,
            replica_groups=[list(range(NCORES))],
            ins=[sendb.opt()], outs=[recvb.opt()])

    # ---- phase D: output projection (t-sharded, full Wo), split by head
    # parity: the even-head half (fed by AllToAll #1) runs while AllToAll
    # #2 is in flight; partials parked in SBUF f32. ----
    po = [pop.tile([128, TS], dtf, name=f"po{oc}") for oc in range(KC)]
    for half, (recvb, wohalf) in enumerate(((recv1, woE), (recv2, woO))):
        ctx_t = []
        for j in range(HG):
            t_ = ccp.tile([128, TS], dtb, tag="cc", name="ctx")
            nc.sync.dma_start(t_[:], recvb[j * 128:(j + 1) * 128, :])
            ctx_t.append(t_)
        for oc in range(KC):
            wo_t = wop.tile([128, HG * 128], dtb, name="wo_t")
            # Act HWDGE queue: independent of the A2A-gated ctx DMAs on the
            # SP queue (no head-of-line blocking).
            nc.scalar.dma_start(
                wo_t[:].rearrange("p (j m) -> p j m", j=HG),
                wohalf[:, oc * 128:(oc + 1) * 128].rearrange(
                    "(j p) m -> p j m", p=128))
            ps_o = psp.tile([128, TS], dtf, tag="mm", name="ps_o")
            for j in range(HG):
                nc.tensor.matmul(
                    ps_o[:], wo_t[:, j * 128:(j + 1) * 128], ctx_t[j][:],
                    start=(j == 0), stop=(j == HG - 1))
            if half == 0:
                nc.scalar.copy(po[oc][:], ps_o[:])
            else:
                ot = otp.tile([128, TS], dtf, name="ot")
                nc.vector.tensor_add(ot[:], ps_o[:], po[oc][:])
                nc.sync.dma_start(outp[oc * 128:(oc + 1) * 128, :], ot[:])


def host_prep(x, Wq, Wk, Wv, Wo):
    bf = ml_dtypes.bfloat16
    x = np.asarray(x, dtype=np.float32)
    Wq = np.asarray(Wq, dtype=np.float32)
    Wk = np.asarray(Wk, dtype=np.float32)
    Wv = np.asarray(Wv, dtype=np.float32)
    Wo = np.asarray(Wo, dtype=np.float32)

    xT = np.ascontiguousarray(x.reshape(BT, D).T).astype(bf)
    perm = np.concatenate([np.arange(0, DH, 2), np.arange(1, DH, 2)])

    pos = np.arange(T, dtype=np.float64)
    inv = ROPE_THETA ** (-np.arange(0, DH, 2, dtype=np.float64) / DH)  # [64]
    ang = inv[:, None] * pos[None, :]                                  # [64, T]
    c64 = np.cos(ang)
    s64 = np.sin(ang)
    cos128 = np.concatenate([c64, c64], axis=0).astype(np.float16)   # [128, T]
    sin128 = np.concatenate([-s64, s64], axis=0).astype(np.float16)  # [-sin; sin]

    tl = np.arange(128)[:, None]
    rl = np.arange(TS)[None, :]
    msk = np.concatenate(
        [(tl + 128 * j <= rl).astype(np.float32) for j in range(4)],
        axis=1).astype(bf)                                     # [128, 4*TS]

    ones = np.ones((128, 128), dtype=bf)
    pswap = np.zeros((128, 128), dtype=np.float32)
    pswap[(np.arange(128) + 64) % 128, np.arange(128)] = 1.0
    pswap = pswap.astype(bf)

    woE = np.ascontiguousarray(
        np.concatenate([Wo[(2 * j) * DH:(2 * j + 1) * DH] for j in range(HG)],
                       axis=0)).astype(bf)
    woO = np.ascontiguousarray(
        np.concatenate([Wo[(2 * j + 1) * DH:(2 * j + 2) * DH] for j in range(HG)],
                       axis=0)).astype(bf)

    in_maps = []
    for i in range(NCORES):
        idx = np.concatenate([i * DC + h * DH + perm for h in range(HPC)])
        in_maps.append({
            "xT": xT,
            "wq": np.ascontiguousarray(Wq[:, idx]).astype(bf),
            "wk": np.ascontiguousarray(Wk[:, idx]).astype(bf),
            "wv": np.ascontiguousarray(Wv[:, i * DC:(i + 1) * DC]).astype(bf),
            "woE": woE, "woO": woO,
            "cosd": cos128, "sind": sin128, "mskd": msk,
            "onesd": ones, "pswapd": pswap,
        })
    return in_maps


def assemble(results):
    out_T = np.concatenate([results[i]["out"] for i in range(NCORES)], axis=1)
    return np.ascontiguousarray(out_T.T).reshape(B, T, D).astype(np.float32)


def kernel(x, Wq, Wk, Wv, Wo):
    nc = build()
    in_maps = host_prep(x, Wq, Wk, Wv, Wo)
    r = bass_utils.run_bass_kernel_spmd(nc, in_maps,
                                        core_ids=list(range(NCORES)))
    return assemble(r.results)
